# revision 25
# baseline (speedup 1.0000x reference)
"""Trainium2 kernel for the quantum-autoencoder forward pass (nn_AutoEncoder).

Math: the circuit uses only RX and CNOT gates on 8 data qubits (the 2 extra
trash-reference wires and the SWAP-test aux wire stay |0> until measurement).
Conjugating by H^x8 turns every RX into a diagonal RZ and every CNOT into a
basis permutation, so in the X-basis the state is always uniform-magnitude:
psi(x) = (1/16) e^{i theta(x)}, theta(x) = sum_g (t_g/2)(2<m_g,x> - 1) with
GF(2)^8 masks m_g evolved through the CNOT network.

The SWAP test gives p1 = (1 - P00)/2 with P00 = prob(trash wires 6,7 = |00>),
and in the X-frame P00 = (1/4)[1 + sum_{e in {e6,e7,e6^e7}} (1/256) *
sum_x cos(D_e(x))], D_e(x) = sum_{g:<m_g,e>=1} t_g (1 - 2<m_g,x>).

Flattened: p1[b] = 3/8 - (1/2048) * sum_{j<768} cos((A @ f_b)_j + (Pw @ w)_j)
with constant sign matrices A (768x8), Pw (768x32) from the circuit wiring.

Structure exploited: up to global row sign (cos is even) the 768 rows of
[A|Pw] collapse to 384 distinct rows sharing only 12 distinct A-patterns.
With U_k = (A_dist f)_k and ctil_r = (Pw_dist w)_r:
    sum_j cos(D_j) = 2 * sum_k [ C_k cos(U_k) - S_k sin(U_k) ]
                   = 2 * sum_k R_k cos(U_k + delta_k),
    C_k = sum_{r in grp k} cos(ctil_r),  S_k = sum_{r in grp k} sin(ctil_r),
    R_k = hypot(C_k, S_k),  delta_k = atan2(S_k, C_k).
The weights path (R_k, delta_k -- 24 numbers from the 32 tiny replicated
weight angles) is evaluated on the host with exact trig; delta_k rides the
ones-row of the matmul table and R_k rides a [128, 9] coefficient image.
The batch-parallel features path (the actual workload: 13 trig terms x 4096
samples) runs on device: p1[b] = 3/8 - (1/1024) sum_k R_k cos(2pi W_k),
W_k = V_k + 1/4 + delta_k/2pi in turns, V = (A_dist f)/2pi.

Device layout (per core, 512 batch rows, pure data parallel on 8 cores):
batch on the FREE axis, terms on partitions, 8 batch groups of 64 with 16
partitions per group (13 used: 12 phased-cos terms + 1 constant that turns
the 3/8 affine into a matmul row).  One fp16 K=65 matmul (block-diagonal
lhsT; fp16 runs the PE at 1 cycle/row and halves the input DMA) produces
W [8 x 16-block, 64 batch] in turns; two DVE ops range-reduce with the fp32
magic-number trick (t = W + 1.5*2^23 rounds to the nearest integer in the
upper mantissa bits; mr = (t - M) - W = k - W in [-0.5, 0.5]; sin(2pi W) =
sin(-2pi mr)); one Sin activation evaluates all terms; one matmul against
the R-coefficient table contracts the 128 term-partitions into p [8, 64]
(group-major -- the scatter below writes DRAM in plain batch order).

Output path: a dma_scatter_add with prepare_only=True generates the
outbound DMA descriptors on the Pool engine's SWDGE ring during the input
DMA / compute window; when the [8, 64] result lands in SBUF, trigger_dma
fires the pre-built descriptors into the pre-zeroed ExternalOutput buffer.
This drops the post-compute tail to trigger + transfer + DMA-sem
propagation, skipping the 625ns HWDGE descriptor-generation and 650ns DGE
delay a plain DMA pays after the data is ready.  The prep's completion
update is rewritten to the tile-assigned DMASW lane semaphore (the slot
bass_interp documents as "the prep's DMASW sem (OnUpdate[0])") so the
end-of-context drain observes the DMA -- tile's sem-assignment pass does
not thread its lane sem through gen_mode=1 preps on its own.
"""

import math
from contextlib import ExitStack

import numpy as np

import concourse.tile as tile
from concourse import bacc, mybir
from concourse.bass_utils import run_bass_kernel_spmd

N_QUBITS = 8
DEPTH = 4
NW = DEPTH * N_QUBITS             # 32 weight angles
BATCH = 4096
N_CORES = 8
SHARD = BATCH // N_CORES          # 512 rows per core
P = 128                           # SBUF partitions
GROUPS = 8                        # batch groups per core
B = SHARD // GROUPS               # 64 batch rows per group
K12 = 12                          # distinct A-patterns
TBLK = 16                         # term-block partitions per group (13 used)
F32 = mybir.dt.float32
F16 = mybir.dt.float16
F32R = mybir.dt.float32r
I16 = mybir.dt.int16
MAGIC = float(1.5 * 2**23)        # fp32 round-to-nearest-integer constant
TWO_PI_GUARD = 2.0 * math.pi * (1.0 - 2.0**-21)  # keep sin arg inside (-pi, pi)
NIDX = 8                          # scatter idx slots (tokens 0..7)


def _build_raw_tables():
    """Phase-tracking masks for the fixed circuit -> sign matrices A, Pw."""
    gates = []  # [mask, ('f'|'w', index)]
    for w in range(N_QUBITS):
        gates.append([1 << w, ("f", w)])
    for l in range(DEPTH):
        for w in range(N_QUBITS):
            gates.append([1 << w, ("w", l * N_QUBITS + w)])
        for w in range(N_QUBITS):
            # original CNOT(ctrl=w, tgt=w+1) -> X-frame ctrl=w+1, tgt=w:
            # masks with bit w set get bit (w+1)%8 flipped
            t, c = w, (w + 1) % N_QUBITS
            for g in gates:
                if g[0] & (1 << t):
                    g[0] ^= 1 << c
    par = np.array([bin(i).count("1") & 1 for i in range(256)], np.int64)
    variants = [1 << 6, 1 << 7, (1 << 6) | (1 << 7)]
    A = np.zeros((3 * 256, N_QUBITS), np.float64)
    Pw = np.zeros((3 * 256, NW), np.float64)
    x = np.arange(256)
    for vi, e in enumerate(variants):
        rows = slice(vi * 256, (vi + 1) * 256)
        for m, (kind, idx) in gates:
            if par[m & e]:
                sigma = 1.0 - 2.0 * par[m & x]
                if kind == "f":
                    A[rows, idx] += sigma
                else:
                    Pw[rows, idx] += sigma
    return A, Pw


def _build_tables():
    A, Pw = _build_raw_tables()
    AB = np.concatenate([A, Pw], axis=1)  # (768, 40)
    # canonicalize row sign by leading nonzero (always in the A part)
    canon = []
    for r in AB:
        nz = np.nonzero(r)[0]
        s = 1.0 if r[nz[0]] > 0 else -1.0
        canon.append(tuple((s * r).tolist()))
    uniq = {}
    for c in canon:
        uniq[c] = uniq.get(c, 0) + 1
    assert len(uniq) == 384 and all(v == 2 for v in uniq.values())
    rows = np.array(list(uniq.keys()))          # (384, 40)
    a_rows = rows[:, :N_QUBITS]                 # (384, 8)
    pw_rows = rows[:, N_QUBITS:]                # (384, 32)
    a_uniq = {}
    for ar in map(tuple, a_rows):
        if ar not in a_uniq:
            a_uniq[ar] = len(a_uniq)
    assert len(a_uniq) == K12
    grp = np.array([a_uniq[tuple(ar)] for ar in a_rows])  # (384,)
    a_dist = np.array(list(a_uniq.keys()))                # (12, 8)

    # Static part of the block-diagonal lhsT [65, 128]: group g's 12 phased
    # terms at cols 16g:16g+12 contract rows 8g:8g+8 (its features); col
    # 16g+12 is the constant term.  Row 64 (ones row, runtime: 1/4 +
    # delta/2pi phase biases) is filled per call in _host_fw_image.
    ad8 = np.zeros((8 * GROUPS + 1, P), np.float64)
    for g in range(GROUPS):
        ad8[8 * g : 8 * g + N_QUBITS, TBLK * g : TBLK * g + K12] = a_dist.T / (
            2 * math.pi
        )
    return ad8, pw_rows, grp


_AD8, _PW_ROWS, _GRP = _build_tables()
_FWROWS = 8 * GROUPS + 1   # 65 contraction rows
_FWCOLS = 256              # 128 lhsT | 64 rhs | pad to 512B descriptor rows
_WVCOLS = 9                # 8 weight columns | 1 int16-packed idx column

# scatter idx table: token t (partition t) -> out row t for t<8, else -1
# (ignored).  Pattern wrapped in 16 and replicated across 128 partitions.
_IDXVALS = np.array(
    [(p % 16) if (p % 16) < 8 else -1 for p in range(P)], np.int16
)


def _host_weight_terms(weights: np.ndarray):
    """Exact weights-path collapse: R_k, delta_k for the 12 term groups."""
    ctil = _PW_ROWS @ weights.astype(np.float64)          # (384,)
    C = np.zeros(K12)
    S = np.zeros(K12)
    for k in range(K12):
        C[k] = np.cos(ctil[_GRP == k]).sum()
        S[k] = np.sin(ctil[_GRP == k]).sum()
    R = np.hypot(C, S)
    delta = np.arctan2(S, C)
    return R, delta


def _host_fw_image(features: np.ndarray, delta: np.ndarray) -> np.ndarray:
    """Per-core [65, 256] fp16 image in matmul orientation: cols 0:128 =
    lhsT (static AD8 + runtime phase ones-row), cols 128:192 = transposed
    feature blocks (row 8g+w = feature w of batch group g; row 64 ones)."""
    feats = features.reshape(N_CORES, GROUPS, B, N_QUBITS)
    img = np.zeros((N_CORES, _FWROWS, _FWCOLS), np.float16)
    img[:, :, :P] = _AD8[None].astype(np.float16)
    ones = np.zeros(P)
    for g in range(GROUPS):
        ones[TBLK * g : TBLK * g + K12] = 0.25 + delta / (2 * math.pi)
        ones[TBLK * g + K12] = 0.25
    img[:, _FWROWS - 1, :P] = ones.astype(np.float16)
    for g in range(GROUPS):
        r = 8 * g
        img[:, r : r + N_QUBITS, P : P + B] = feats[:, g].transpose(0, 2, 1)
    img[:, _FWROWS - 1, P : P + B] = 1.0
    return img


def _host_wv_image(R: np.ndarray) -> np.ndarray:
    """[128, 9] f32 image: col g (g<8) carries group g's 13-term weights at
    rows 16g:16g+13 = [-R_k/1024 | 3/8]; col 8 packs the int16 scatter idx
    table into the low bytes of each f32 word."""
    wv = np.zeros((P, _WVCOLS), np.float32)
    for g in range(GROUPS):
        wv[TBLK * g : TBLK * g + K12, g] = -R / 1024.0
        wv[TBLK * g + K12, g] = 3.0 / 8.0
    pair = np.zeros((P, 2), np.int16)
    pair[:, 0] = _IDXVALS
    wv[:, 8] = pair.view(np.float32)[:, 0]
    return wv


def _patch_prep_dmasw(nc):
    """Point each prep's descriptor-completion update (OnUpdate[0] -- the
    slot bass_interp calls 'the prep's DMASW sem') at its tile-assigned
    DMASW lane semaphore, so the end-of-context drain's DMASW waits are
    satisfied when the triggered DMAs land.  Tile assigns lanes to Pool DMA
    instructions in scheduled (block) order, matching the prep order here."""
    fn = nc.m.functions[0]
    lanes = {}
    preps = []
    for b in fn.blocks:
        for inst in b.instructions:
            si = inst.sync_info
            if si is None:
                continue
            if getattr(inst, "gen_mode", 0) == 1:
                preps.append(inst)
            for w in si.on_wait:
                if w.ant_name and w.ant_name.startswith("DMASW"):
                    lanes[w.ant_name] = w.id
    lane_list = sorted(lanes.items())  # DMASW0_x < DMASW1_x
    assert len(lane_list) == len(preps), (lane_list, [p.name for p in preps])
    for inst, (lname, lid) in zip(preps, lane_list):
        si = inst.sync_info
        old = si.on_update[0]
        si.on_update[0] = mybir.SyncUpdate(
            sync_type="semaphore",
            id=lid,
            ant_name=lname,
            update_mode=old.update_mode,
            update_value=16,
            update_reg=None,
        )


_CACHE = {}


def _build_nc():
    nc = bacc.Bacc(
        "TRN2",
        target_bir_lowering=False,
        debug=False,
        num_devices=N_CORES,
    )
    fw = nc.dram_tensor("fw", [_FWROWS, _FWCOLS], F16, kind="ExternalInput")
    wvd = nc.dram_tensor("wv", [P, _WVCOLS], F32R, kind="ExternalInput")
    # out[g, b] = batch row g*64+b of the shard (plain batch-major order)
    out = nc.dram_tensor("out", [GROUPS, B], F32, kind="ExternalOutput")

    SIN = mybir.ActivationFunctionType.Sin
    SUB = mybir.AluOpType.subtract

    with tile.TileContext(nc) as tc, ExitStack() as ctx:
        const = ctx.enter_context(tc.tile_pool(name="const", bufs=1))
        work = ctx.enter_context(tc.tile_pool(name="work", bufs=2))
        vps = ctx.enter_context(tc.tile_pool(name="vpsum", bufs=1, space="PSUM"))

        one_c = nc.const_aps.tensor(1.0, (1, 1))

        # dummy Sin first: triggers the ACT table load at t=0 so it overlaps
        # the input DMA instead of sitting on the critical path
        dummy = const.tile([1, 1], F32)
        nc.scalar.activation(dummy[:], one_c, SIN, bias=0.0, scale=0.0)

        # critical input image on the SP HWDGE queue (a SWDGE gather-prep
        # alternative measures ~110ns slower: Q7 library reloads + iota +
        # the 1016ns desc-gen outweigh the skipped HWDGE+DGE stages); the
        # weight/idx image follows on the same queue (completes ~1.2us
        # before first use)
        f_s = const.tile([_FWROWS, _FWCOLS], F16)
        nc.sync.dma_start(f_s[:], fw.ap()[:])
        wv_s = const.tile([P, _WVCOLS], F32R)
        nc.sync.dma_start(wv_s[:], wvd.ap()[:])

        # scatter source [128 tokens, 1, 64]; fully memset early so the
        # triggered DMA never reads uninitialized SBUF (only partitions 0:8
        # carry data; idx -1 ignores the rest)
        res = const.tile([P, 1, B], F32)
        nc.vector.memset(res[:], 0.0)

        # W [8 x 16-term blocks, 64 batch] in turns (incl. 1/4 + delta/2pi
        # phase biases from the ones-row)
        v_p = vps.tile([P, B], F32, tag="v")
        nc.tensor.matmul(
            v_p[:], f_s[:, :P], f_s[:, P : P + B], start=True, stop=True
        )

        # range reduction (two DVE ops: the hardware forbids a single op
        # reading two PSUM tensors, a single op cannot read v twice, and
        # GPSIMD instructions cannot access PSUM -- the BIR verifier
        # rejects the otherwise-tempting Pool-engine variant)
        t_s = work.tile([P, B], F32, tag="t")
        nc.vector.tensor_scalar_add(t_s[:], v_p[:], MAGIC)
        mr_s = vps.tile([P, B], F32, tag="mr")
        nc.vector.scalar_tensor_tensor(
            mr_s[:], t_s[:], MAGIC, v_p[:], op0=SUB, op1=SUB
        )

        # big Sin: sv = sin(-2pi * mr) = sin(2pi * W) = cos-with-phase terms
        sv_s = work.tile([P, B], F32R, tag="sv")
        nc.scalar.activation(
            sv_s[:], mr_s[:], SIN, bias=0.0, scale=-TWO_PI_GUARD
        )

        # weighted sum over the 128 term-partitions: p [8 groups, 64 batch]
        p_p = vps.tile([GROUPS, B], F32, tag="p")
        nc.tensor.matmul(
            p_p[:], wv_s[:, :GROUPS], sv_s[:], start=True, stop=True
        )
        nc.vector.tensor_copy(res[:GROUPS, 0, :], p_p[:])

        # prepare_only scatter: descriptors are generated on the Pool engine
        # as soon as the idx table lands (wv DMA), long before the data;
        # the trigger inherits the RAW dep on res and fires on the copy's
        # completion sem.  idx: token t -> out row t (t<8), -1 ignored.
        dma_sem = nc.alloc_semaphore("wb_dma")
        idx_ap = wv_s[:, 8:9].bitcast(I16)[:, 0:1]
        nc.gpsimd.dma_scatter_add(
            out.ap()[:],
            res[:],
            idx_ap,
            NIDX,
            GROUPS,
            B,
            prepare_only=True,
            sem=dma_sem,
        )
        nc.gpsimd.trigger_dma(count=None)

    _patch_prep_dmasw(nc)
    nc.compile()
    return nc


def get_nc():
    if "nc" not in _CACHE:
        _CACHE["nc"] = _build_nc()
    return _CACHE["nc"]


def kernel(features: np.ndarray, weights: np.ndarray, **run_kwargs) -> np.ndarray:
    nc = get_nc()
    R, delta = _host_weight_terms(np.ascontiguousarray(weights, np.float64))
    fw = _host_fw_image(np.ascontiguousarray(features, np.float32), delta)
    wv = _host_wv_image(R)
    in_maps = [{"fw": fw[i], "wv": wv} for i in range(N_CORES)]
    last_err = None
    for attempt in range(3):
        try:
            r = run_bass_kernel_spmd(
                nc, in_maps, core_ids=list(range(N_CORES)), **run_kwargs
            )
            break
        except Exception as e:  # transient device-unrecoverable states
            last_err = e
            if attempt == 2:
                raise
            import time

            time.sleep(45)
    out = np.concatenate(
        [np.asarray(r.results[i]["out"]).reshape(SHARD) for i in range(N_CORES)]
    )
    if run_kwargs:
        return out.astype(np.float32), r
    return out.astype(np.float32)
